# revision 15
# baseline (speedup 1.0000x reference)
"""MLA multi-head latent attention kernel for 8 TRN2 NeuronCores.

Sharding: 8 cores = 2 batches (DP) x 4 head-groups of 4 heads (TP).
The shared LoRA down-projection (x @ [q_a|kv_a]) is token-split across the
4 cores of each batch group (each computes its own 512-token chunk) and the
bf16 activations are AllGathered within the group — no replicated phase-1
work.  Three staggered gathers — [kv | q0], [q1..6], [q7..11] — overlap the
NRT collective latency (~35us trigger + serialization per op) with the
remaining phase-1 / kv_b / partial-q_b compute: q chunk 0 rides the kv
gather's mandatory slot, and each successive gather extends the q_b
accumulation chains (k<1, then k<7) so only ~5 contraction steps per chain
remain when the last gather lands.  A dummy warmup collective at kernel
start absorbs comm bring-up.

Key device facts this kernel is shaped around: the PE has DVFS p-states
(2.4 GHz only after ~3us of continuous busy, 1.2 GHz after any stall), so
every phase is software-pipelined to keep the PE gap-free; vector
reciprocal is ~16x slow, so 1/x and 1/sqrt run as scalar exp(-ln) /
exp(-0.5 ln); DMA-engine contention delays eviction DMAs (and therefore
collective triggers), so w1 is host-pre-arranged into contiguous per-chunk
tiles and later-phase weights prefetch only after the phase-1 stream.

On-device everything is feature-major ([feature, T]); no activation
transposes anywhere:
  - host supplies x pre-transposed per chunk (xT [D, 512]) and weights
    pre-sliced, with RMS-norm weights and softmax scale folded into the
    up-projections
  - kv_c is RMS-normalized locally in phase 1 (exact f32 sum-of-squares off
    PSUM via ones-matmul) and gathered pre-scaled, so kv_b needs no norm
    handling downstream; q is gathered raw and its norm recomputed per
    chunk (ones-matmul trick), applied at PSUM-eviction time
  - q up-projection outputs stay SBUF-resident ([768, T] bf16) through
    attention (no DRAM staging)
  - attention computes scores transposed ([k, q]) with a lookahead-2
    software pipeline (scores of tile i+2 issue before attn@v of tile i, so
    the PE never waits on the scalar exp); the softmax-sum accumulates on
    the vector engine (frees a PSUM bank and one PE pass per tile) with a
    single ones-matmul partition-reduce per head; exp runs without
    max-subtraction (scores are small by construction)
  - o_proj for chunk qn-1 is emitted during attention of chunk qn (absorbs
    eviction latency at chunk boundaries); partial outputs are written f16
    ([D, T]) and summed on host
Measured: 619 us on hardware under a heavily power-throttled device state
(util limit 0.68; the prior 3-gather structure measured 627-694 us under
comparable throttle; baseline replicated-phase-1 version was 702 us),
rel err 5.9e-3 vs the fp32 reference.
"""
import math
import sys
from collections import deque
from contextlib import ExitStack
from dataclasses import dataclass

sys.path.insert(0, '/opt/trn_rl_repo')
import numpy as np
import ml_dtypes
import concourse.bass as bass
import concourse.bacc as bacc
import concourse.mybir as mybir
from concourse import tile
from concourse.bass_utils import run_bass_kernel_spmd

F32 = mybir.dt.float32
F16 = mybir.dt.float16
BF16 = mybir.dt.bfloat16
AF = mybir.ActivationFunctionType


@dataclass
class Cfg:
    T: int = 2048
    D: int = 2048
    QL: int = 1536
    KVL: int = 512
    NHC: int = 4          # heads per core
    NOPE: int = 128
    ROPE: int = 64
    V: int = 128
    G: int = 4            # cores per TP group = token chunks
    eps: float = 1e-6
    rope_base: float = 10000.0

    @property
    def TC(self):         # tokens computed locally in phase 1
        return self.T // self.G

    @property
    def NC(self):
        return min(512, self.T)

    @property
    def KD(self):
        return self.D // 128

    @property
    def MQ(self):
        return self.QL // 128

    @property
    def MKV(self):
        return self.KVL // 128

    @property
    def NT(self):
        return self.T // self.NC

    @property
    def TK(self):
        return self.T // 128

    @property
    def MASKW(self):
        return 2 * self.NC - 128

    @property
    def QH(self):
        return self.NOPE + self.ROPE

    @property
    def NQM(self):        # q_b output 128-chunks
        return (self.NHC * self.QH) // 128

    @property
    def KVC(self):        # kv_a output rows (latent + rope)
        return self.KVL + self.ROPE


# full-scale problem constants (per harness contract)
B, T, D = 2, 2048, 2048
QL, KVL = 1536, 512
NHEADS, NOPE, ROPE, V = 16, 128, 64, 128
QH = NOPE + ROPE
NCORES = 8
GROUPS = 4
NHC = NHEADS // GROUPS
FULL = Cfg()
RG = [[0, 1, 2, 3], [4, 5, 6, 7]]
MQ1 = 0               # q chunks riding in the first (kv) gather: none — the
                      # kv gather must trigger as soon as kv+rope evict (~35us)
                      # since the first collective pays ~45us NRT bring-up
MQ2 = 7               # first q chunk of the third gather


def _r(ap):
    return ap


def build_nc(c: Cfg = FULL, num_devices: int = NCORES):
    nc = bacc.Bacc("TRN2", target_bir_lowering=False, debug=False,
                   num_devices=num_devices)
    W1C = c.QL + c.KVL + c.ROPE
    TC = c.TC

    NM1 = (W1C + 127) // 128
    xT = nc.dram_tensor("xT", [c.D, TC], BF16, kind="ExternalInput").ap()
    # w1 pre-arranged on host: chunk mi's [128, KD*128] tile is contiguous
    w1 = nc.dram_tensor("w1", [NM1 * 128, c.KD * 128], BF16,
                        kind="ExternalInput").ap()
    qbw = nc.dram_tensor("qbw", [c.QL, c.NHC * c.QH], BF16, kind="ExternalInput").ap()
    kbw = nc.dram_tensor("kbw", [c.KVL, c.NHC * 128], BF16, kind="ExternalInput").ap()
    vbw = nc.dram_tensor("vbw", [c.KVL, c.NHC * c.V], BF16, kind="ExternalInput").ap()
    ow = nc.dram_tensor("ow", [c.NHC * c.V, c.D], BF16, kind="ExternalInput").ap()
    cos2 = nc.dram_tensor("cos2", [128, c.T], BF16, kind="ExternalInput").ap()
    sin2 = nc.dram_tensor("sin2", [128, c.T], BF16, kind="ExternalInput").ap()
    maskt = nc.dram_tensor("maskt", [128, c.MASKW], F32, kind="ExternalInput").ap()
    outT = nc.dram_tensor("outT", [c.D, c.T], F16, kind="ExternalOutput").ap()

    m1 = []
    off = 0
    while off < W1C:
        sz = min(128, W1C - off)
        m1.append((off, sz))
        off += sz
    m_order = list(range(c.MQ, len(m1))) + list(range(c.MQ))  # kv chunks first

    with tile.TileContext(nc) as tc, ExitStack() as top:
        # gather 1 carries kv + the first MQ1 q chunks (asymmetric split:
        # the chain [warmup -> kv+q0..2 -> q3..11] finishes ~30us earlier
        # than [kv -> q0..5 -> q6..11] and the k<MQ1 partial q_b chains fill
        # the second transfer window)
        KQ1 = c.KVC + MQ1 * 128
        dram = top.enter_context(tc.tile_pool(name="dram", bufs=1, space="DRAM"))
        # each DRAM tile gets its own tag: untagged tiles share one tag-meta
        # rotation in the pool, which serializes later tiles (and the gathers
        # writing them) behind every reader of the earlier ones
        kq_loc = dram.tile([KQ1, TC], BF16, tag="kq_loc")
        q2_loc = dram.tile([(MQ2 - MQ1) * 128, TC], BF16, tag="q2_loc")
        q3_loc = dram.tile([(c.MQ - MQ2) * 128, TC], BF16, tag="q3_loc")
        kqg = dram.tile([c.NT * KQ1, TC], BF16, tag="kqg")
        qg2 = dram.tile([c.NT * (MQ2 - MQ1) * 128, TC], BF16, tag="qg2")
        qg3 = dram.tile([c.NT * (c.MQ - MQ2) * 128, TC], BF16, tag="qg3")
        # (no warmup collective: the NRT start barrier covers comm bring-up,
        # and a warmup op only serializes ahead of the first real gather)

        const = top.enter_context(tc.tile_pool(name="const", bufs=1))
        ones_f = const.tile([128, 128], F32)
        nc.vector.memset(ones_f[:], 1.0)
        ones = const.tile([128, 128], BF16)
        nc.vector.tensor_copy(ones[:], ones_f[:])
        eps_sb = const.tile([128, 1], F32)
        nc.vector.memset(eps_sb[:], float(c.eps))
        cos_sb = const.tile([128, c.T], BF16, tag="cos")
        sin_sb = const.tile([128, c.T], BF16, tag="sin")
        # later-phase weight pools opened early: their loads are issued
        # mid-phase-1 (after the kv gather fires) so the gather windows have
        # the DMA engines to themselves
        kw_pool = top.enter_context(tc.tile_pool(name="kw", bufs=1))
        kbw_sb = [kw_pool.tile([128, c.NHC * 128], BF16, tag=f"kbw{k}",
                               name=f"kbw{k}") for k in range(c.MKV)]
        vbw_sb = [kw_pool.tile([128, c.NHC * c.V], BF16, tag=f"vbw{k}",
                               name=f"vbw{k}") for k in range(c.MKV)]
        qw_pool = top.enter_context(tc.tile_pool(name="qw", bufs=1))
        qbw_sb = [qw_pool.tile([128, c.NHC * c.QH], BF16, tag=f"qbw{k}",
                               name=f"qbw{k}") for k in range(c.MQ)]
        mk_pool = top.enter_context(tc.tile_pool(name="mask", bufs=1))
        mask_sb = mk_pool.tile([128, c.MASKW], F32)

        kvc = top.enter_context(tc.tile_pool(name="kvc", bufs=1))  # "KV cache"
        knope = [kvc.tile([128, c.T], BF16, tag=f"kn{i}", name=f"kn{i}")
                 for i in range(c.NHC)]
        krope = kvc.tile([128, c.T], BF16, tag="krope")  # duplicated halves
        vsb = [kvc.tile([128, c.NHC * c.V], BF16, tag=f"v{i}", name=f"v{i}")
               for i in range(c.TK)]
        qsb = [kvc.tile([128, c.T], BF16, tag=f"q{m}", name=f"qs{m}")
               for m in range(c.NQM)]   # q up-proj output, SBUF-resident

        # ---------------- phase 1: local-chunk x @ [q_a | kv_a | k_rope] ----
        with ExitStack() as p1:
            xt_pool = p1.enter_context(tc.tile_pool(name="xt", bufs=1))
            w1_pool = p1.enter_context(tc.tile_pool(name="w1", bufs=2))
            ev_pool = p1.enter_context(tc.tile_pool(name="p1ev", bufs=18))
            ps_pool = p1.enter_context(tc.tile_pool(name="p1ps", bufs=2, space="PSUM"))
            kssq_ps = p1.enter_context(tc.tile_pool(name="kssq", bufs=1, space="PSUM"))
            krs_pool = p1.enter_context(tc.tile_pool(name="krs", bufs=1))

            qssq_ps = p1.enter_context(tc.tile_pool(name="qssq", bufs=1,
                                                    space="PSUM"))
            xt_sb = [xt_pool.tile([128, TC], BF16, tag=f"xt{k}", name=f"xt{k}")
                     for k in range(c.KD)]

            def load_wt(mi):
                wt = w1_pool.tile([128, c.KD, 128], BF16, tag="w1t")
                src = w1[mi * 128:(mi + 1) * 128, :].rearrange(
                    "p (k c) -> p k c", k=c.KD)
                nc.sync.dma_start(wt[:], src)
                return wt

            # first weight chunk before the xt stream so chain 0 starts early
            wts = {m_order[0]: load_wt(m_order[0])}
            for k in range(c.KD):
                nc.sync.dma_start(xt_sb[k][:], xT[k * 128:(k + 1) * 128, :])
            # kv_c AND q are RMS-normalized locally (exact f32 ssq from PSUM)
            # and gathered pre-scaled, so neither kv_b nor q_b needs any norm
            # handling downstream (the q ssq-recompute used to cost 48 PE MMs
            # + 48 vector muls + 8 Ln/Exp table switches post-gather).
            kv_evs, q_evs = [], []
            kssq = kssq_ps.tile([128, TC], F32, tag="kssq", name="kssq")
            qssq = qssq_ps.tile([128, TC], F32, tag="qssq", name="qssq")
            for pos, mi in enumerate(m_order):
                m0, msz = m1[mi]
                wt = wts.pop(mi) if mi in wts else load_wt(mi)
                ps = ps_pool.tile([128, TC], F32, tag="ps")
                for k in range(c.KD):
                    nc.tensor.matmul(ps[:msz, :], _r(wt[:, k, :msz]),
                                     _r(xt_sb[k][:]),
                                     start=(k == 0), stop=(k == c.KD - 1))
                ev = ev_pool.tile([128, TC], BF16, tag="ev")
                nc.scalar.copy(ev[:msz, :], ps[:msz, :])
                is_kvc = c.MQ <= mi < c.MQ + c.MKV
                if is_kvc:
                    sq = ev_pool.tile([128, TC], BF16, tag="sq1", name="sq1")
                    nc.scalar.square(sq[:], ps[:])
                    nc.tensor.matmul(kssq[:], _r(ones[:]), _r(sq[:]),
                                     start=(mi == c.MQ),
                                     stop=(mi == c.MQ + c.MKV - 1))
                    kv_evs.append((m0 - c.QL, msz, ev))
                    if mi == c.MQ + c.MKV - 1:
                        lt = krs_pool.tile([128, TC], F32, tag="klt", name="klt")
                        krsq = krs_pool.tile([128, TC], F32, tag="krsq",
                                             name="krsq")
                        nc.scalar.activation(lt[:], kssq[:], AF.Ln,
                                             bias=eps_sb[:], scale=1.0 / c.KVL)
                        nc.scalar.activation(krsq[:], lt[:], AF.Exp, scale=-0.5)
                        for (roff, msze, eve) in kv_evs:
                            nc.vector.tensor_mul(eve[:msze, :], eve[:msze, :],
                                                 krsq[:msze, :])
                            nc.sync.dma_start(kq_loc[roff:roff + msze, :],
                                              eve[:msze, :])
                    continue
                if mi >= c.MQ:          # rope chunk: evict raw (no norm)
                    nc.sync.dma_start(kq_loc[m0 - c.QL:m0 - c.QL + msz, :],
                                      ev[:msz, :])
                    # kv + rope fully evicted: fire the kv gather now
                    nc.gpsimd.collective_compute(
                        "AllGather", mybir.AluOpType.bypass,
                        replica_groups=RG,
                        ins=[kq_loc.opt()], outs=[kqg.opt()])
                    # weights needed right after this gather load now,
                    # before the transfer windows occupy the DMA engines
                    nc.sync.dma_start(cos_sb[:], cos2[:])
                    nc.sync.dma_start(sin_sb[:], sin2[:])
                    nc.sync.dma_start(mask_sb[:], maskt[:])
                    for k in range(c.MKV):
                        nc.sync.dma_start(kbw_sb[k][:],
                                          kbw[k * 128:(k + 1) * 128, :])
                        nc.sync.dma_start(vbw_sb[k][:],
                                          vbw[k * 128:(k + 1) * 128, :])
                    continue
                # q chunk: accumulate ssq, hold the eviction until qrsq ready
                sq = ev_pool.tile([128, TC], BF16, tag="sq1", name="sq1")
                nc.scalar.square(sq[:], ps[:])
                nc.tensor.matmul(qssq[:], _r(ones[:]), _r(sq[:]),
                                 start=(mi == 0), stop=(mi == c.MQ - 1))
                q_evs.append((mi, msz, ev))
                if mi != c.MQ - 1:
                    continue
                qlt = krs_pool.tile([128, TC], F32, tag="qlt", name="qlt")
                qrsq = krs_pool.tile([128, TC], F32, tag="qrsq", name="qrsq")
                nc.scalar.activation(qlt[:], qssq[:], AF.Ln,
                                     bias=eps_sb[:], scale=1.0 / c.QL)
                nc.scalar.activation(qrsq[:], qlt[:], AF.Exp, scale=-0.5)
                for (qmi, msze, eve) in q_evs:
                    nc.vector.tensor_mul(eve[:msze, :], eve[:msze, :],
                                         qrsq[:msze, :])
                    if qmi < MQ1:
                        dst, roff = kq_loc, c.KVC + qmi * 128
                    elif qmi < MQ2:
                        dst, roff = q2_loc, (qmi - MQ1) * 128
                    else:
                        dst, roff = q3_loc, (qmi - MQ2) * 128
                    nc.sync.dma_start(dst[roff:roff + msze, :], eve[:msze, :])
                    if qmi == MQ2 - 1:
                        nc.gpsimd.collective_compute(
                            "AllGather", mybir.AluOpType.bypass,
                            replica_groups=RG,
                            ins=[q2_loc.opt()], outs=[qg2.opt()])
                    elif qmi == c.MQ - 1:
                        nc.gpsimd.collective_compute(
                            "AllGather", mybir.AluOpType.bypass,
                            replica_groups=RG,
                            ins=[q3_loc.opt()], outs=[qg3.opt()])
                for k in range(c.MQ):
                    nc.sync.dma_start(qbw_sb[k][:],
                                      qbw[k * 128:(k + 1) * 128, :])

        # ---------------- phases 2a (kv_b) + 2b (q_b) ----------------
        with ExitStack() as mid:
            kvr_pool = mid.enter_context(tc.tile_pool(name="kvr", bufs=1))
            kv_raw = [kvr_pool.tile([128, c.T], BF16, tag=f"kvr{k}", name=f"kvr{k}")
                      for k in range(c.MKV)]
            for n in range(c.NT):
                ns = slice(n * c.NC, (n + 1) * c.NC)
                base = n * KQ1
                for k in range(c.MKV):
                    nc.sync.dma_start(kv_raw[k][:, ns],
                                      kqg[base + k * 128:base + (k + 1) * 128, :])
                nc.sync.dma_start(krope[0:64, ns],
                                  kqg[base + c.KVL:base + c.KVC, :])
                nc.sync.dma_start(krope[64:128, ns],
                                  kqg[base + c.KVL:base + c.KVC, :])
            rt_pool = mid.enter_context(tc.tile_pool(name="rt", bufs=2))
            ps2 = mid.enter_context(tc.tile_pool(name="ps2", bufs=2, space="PSUM"))

            def rope_cols(x_ap, rows, ns):
                # in-place rotate-half on [rows, NC] slice; tables sliced to ns
                tmp = rt_pool.tile([128, c.NC], BF16, tag="rtmp", name="rtmp")
                t1 = rt_pool.tile([128, c.NC], BF16, tag="rt1", name="rt1")
                for b0 in range(0, rows, 64):
                    nc.sync.dma_start(tmp[b0:b0 + 32, :], x_ap[b0 + 32:b0 + 64, :])
                    nc.sync.dma_start(tmp[b0 + 32:b0 + 64, :], x_ap[b0:b0 + 32, :])
                nc.vector.tensor_mul(tmp[:rows, :], tmp[:rows, :],
                                     sin_sb[:rows, ns])
                nc.vector.tensor_mul(t1[:rows, :], x_ap[:rows, :],
                                     cos_sb[:rows, ns])
                nc.vector.tensor_add(x_ap[:rows, :], t1[:rows, :], tmp[:rows, :])

            for n in range(c.NT):
                ns = slice(n * c.NC, (n + 1) * c.NC)
                rope_cols(krope[:, ns], 128, ns)

            # gathered kv_c is already RMS-normalized: plain evictions
            for h in range(c.NHC):
                for n in range(c.NT):
                    ns = slice(n * c.NC, (n + 1) * c.NC)
                    ps = ps2.tile([128, c.NC], F32, tag="ps")
                    for k in range(c.MKV):
                        nc.tensor.matmul(ps[:], _r(kbw_sb[k][:, h * 128:(h + 1) * 128]),
                                         _r(kv_raw[k][:, ns]),
                                         start=(k == 0), stop=(k == c.MKV - 1))
                    nc.vector.tensor_copy(knope[h][:, ns], ps[:])

            for m in range(c.TK):
                ms = slice(m * 128, (m + 1) * 128)
                ps = ps2.tile([128, c.NC], F32, tag="ps")
                for k in range(c.MKV):
                    nc.tensor.matmul(ps[:], _r(kv_raw[k][:, ms]),
                                     _r(vbw_sb[k][:]),
                                     start=(k == 0), stop=(k == c.MKV - 1))
                nc.vector.tensor_copy(vsb[m][:], ps[:])

            # ------------ phase 2b: q_b (+ RoPE on q), SBUF-resident --------
            xq_pool = mid.enter_context(tc.tile_pool(name="xq", bufs=2 * c.MQ + 2))
            ps2b = mid.enter_context(tc.tile_pool(name="ps2b", bufs=2, space="PSUM"))

            for n in range(c.NT):
                ns = slice(n * c.NC, (n + 1) * c.NC)
                xq = []
                for k in range(c.MQ):
                    t = xq_pool.tile([128, c.NC], BF16, tag="xq", name=f"xq{k}")
                    if k < MQ1:
                        base = n * KQ1 + c.KVC + k * 128
                        src = kqg[base:base + 128, :]
                    elif k < MQ2:
                        base = n * (MQ2 - MQ1) * 128 + (k - MQ1) * 128
                        src = qg2[base:base + 128, :]
                    else:
                        base = n * (c.MQ - MQ2) * 128 + (k - MQ2) * 128
                        src = qg3[base:base + 128, :]
                    nc.sync.dma_start(t[:], src)
                    xq.append(t)
                # gathered q is already RMS-normalized: plain evictions
                for m in range(c.NQM):
                    ps = ps2b.tile([128, c.NC], F32, tag="ps")
                    for k in range(c.MQ):
                        nc.tensor.matmul(ps[:], _r(qbw_sb[k][:, m * 128:(m + 1) * 128]),
                                         _r(xq[k][:]),
                                         start=(k == 0), stop=(k == c.MQ - 1))
                    nc.vector.tensor_copy(qsb[m][:, ns], ps[:])
                    if m >= c.NHC:
                        rope_cols(qsb[m][:, ns], 128, ns)

        # ---------------- phase 3+4: attention + interleaved o_proj --------
        with ExitStack() as late:
            av_pool = late.enter_context(tc.tile_pool(name="avt", bufs=1))
            avt = [av_pool.tile([128, c.T], BF16, tag=f"av{i}", name=f"av{i}")
                   for i in range(c.NHC)]
            ow_pool = late.enter_context(tc.tile_pool(name="ow", bufs=1))
            ow_sb = [ow_pool.tile([128, c.D], BF16, tag=f"ow{h}", name=f"ow{h}")
                     for h in range(c.NHC)]
            for h in range(c.NHC):
                nc.sync.dma_start(ow_sb[h][:], ow[h * c.V:(h + 1) * c.V, :])

            s_ps = late.enter_context(tc.tile_pool(name="sps", bufs=3, space="PSUM"))
            av_ps = late.enter_context(tc.tile_pool(name="avps", bufs=2, space="PSUM"))
            sm_ps = late.enter_context(tc.tile_pool(name="smps", bufs=1, space="PSUM"))
            ps4 = late.enter_context(tc.tile_pool(name="ps4", bufs=2, space="PSUM"))
            e_pool = late.enter_context(tc.tile_pool(name="e", bufs=6))
            acc_pool = late.enter_context(tc.tile_pool(name="acc", bufs=2))
            rs_pool = late.enter_context(tc.tile_pool(name="rs", bufs=2))
            ev4 = late.enter_context(tc.tile_pool(name="ev4", bufs=3))

            def nkj_of(qn):
                return ((qn + 1) * c.NC) // 128

            def score(qn, h, kj):
                # scores^T for one 128-key tile -> exp'd bf16 e tile
                qsl = slice(qn * c.NC, (qn + 1) * c.NC)
                qr_t = qsb[c.NHC + (h * c.ROPE) // 128]
                qr_r0 = (h * c.ROPE) % 128
                ks = slice(kj * 128, (kj + 1) * 128)
                pss = s_ps.tile([128, c.NC], F32, tag="pss")
                nc.tensor.matmul(pss[:], _r(knope[h][:, ks]),
                                 _r(qsb[h][:, qsl]), start=True, stop=False)
                nc.tensor.matmul(pss[:], _r(krope[qr_r0:qr_r0 + 64, ks]),
                                 _r(qr_t[qr_r0:qr_r0 + 64, qsl]),
                                 start=False, stop=True)
                e = e_pool.tile([128, c.NC], BF16, tag="e")
                off = kj * 128 - qn * c.NC
                if off >= 0:  # diagonal tile: causal mask
                    msl = mask_sb[:, c.NC - 128 - off:2 * c.NC - 128 - off]
                    nc.vector.tensor_add(e[:], pss[:], msl)
                    nc.scalar.activation(e[:], e[:], AF.Exp)
                else:
                    nc.scalar.activation(e[:], pss[:], AF.Exp)
                return e

            cur = {}           # (qn, h) -> (pav psum, acc sbuf)
            key_order = []

            def avsum(qn, h, kj, e):
                # attn@v on PE; softmax-sum accumulated on vector (frees a
                # PSUM bank and 1 PE pass per tile)
                if kj == 0:
                    cur[(qn, h)] = (av_ps.tile([128, c.NC], F32, tag="pav",
                                               name="pav"),
                                    acc_pool.tile([128, c.NC], F32, tag="acc",
                                                  name="acc"))
                    key_order.append((qn, h))
                pav, acc = cur[(qn, h)]
                first, last = (kj == 0), (kj == nkj_of(qn) - 1)
                nc.tensor.matmul(pav[:], _r(vsb[kj][:, h * c.V:(h + 1) * c.V]),
                                 _r(e[:]), start=first, stop=last)
                if first:
                    nc.vector.tensor_copy(acc[:], e[:])
                else:
                    nc.vector.tensor_add(acc[:], acc[:], e[:])

            def finalize(qn, h):
                # partition-reduce the vector-accumulated sum with one
                # ones-matmul, then 1/sum on the vector engine (keeps Ln off
                # the scalar queue: any Ln between attention Exps forces a
                # ~2.7us ACT_TABLE_LOAD pair that stalls the exp->attn@v path)
                qsl = slice(qn * c.NC, (qn + 1) * c.NC)
                pav, acc = cur.pop((qn, h))
                ab = e_pool.tile([128, c.NC], BF16, tag="accb", name="accb")
                nc.vector.tensor_copy(ab[:], acc[:])
                psm = sm_ps.tile([128, c.NC], F32, tag="psm", name="psm")
                nc.tensor.matmul(psm[:], _r(ones[:]), _r(ab[:]),
                                 start=True, stop=True)
                rs = rs_pool.tile([128, c.NC], F32, tag="rs")
                nc.vector.reciprocal_approx_fast(rs[:], psm[:])
                nc.vector.tensor_mul(avt[h][:, qsl], pav[:], rs[:])

            def oproj(qn):
                qsl = slice(qn * c.NC, (qn + 1) * c.NC)
                for m in range(c.KD):
                    ms = slice(m * 128, (m + 1) * 128)
                    ps = ps4.tile([128, c.NC], F32, tag="ps")
                    for h in range(c.NHC):
                        nc.tensor.matmul(ps[:], _r(ow_sb[h][:, ms]),
                                         _r(avt[h][:, qsl]),
                                         start=(h == 0), stop=(h == c.NHC - 1))
                    ev = ev4.tile([128, c.NC], F16, tag="ev")
                    # scalar, not vector: the vector queue is the co-critical
                    # engine during attention (acc adds); ScE reads PSUM fast
                    nc.scalar.copy(ev[:], ps[:])
                    nc.sync.dma_start(outT[ms, qsl], ev[:])

            # software pipeline with lookahead 2: PE issues scores two tiles
            # ahead of attn@v, so the scalar exp latency is fully hidden and
            # the PE stream never bubbles (keeps the DVFS p-state at max).
            # o_proj for chunk qn-1 is emitted during chunk qn to absorb the
            # eviction latency at chunk boundaries.
            pend = deque()
            done_keys = 0

            def pop_one():
                nonlocal done_keys
                (pqn, ph, pkj, pe) = pend.popleft()
                avsum(pqn, ph, pkj, pe)
                # finalize the previous head once the next head is 4 tiles in
                # (gives the vector queue time to drain the sum accumulation)
                if pkj == 3 and done_keys < len(key_order) - 1:
                    finalize(*key_order[done_keys])
                    done_keys += 1

            for qn in range(c.NT):
                for h in range(c.NHC):
                    for kj in range(nkj_of(qn)):
                        e = score(qn, h, kj)
                        pend.append((qn, h, kj, e))
                        if len(pend) > 3:
                            pop_one()
                if qn > 0:
                    oproj(qn - 1)
            while pend:
                pop_one()
            while done_keys < len(key_order):
                finalize(*key_order[done_keys])
                done_keys += 1
            oproj(c.NT - 1)

    nc.compile()
    return nc


# ---------------- host-side prep ----------------
def make_tables(c: Cfg):
    j = np.arange(32, dtype=np.float64)
    invf = c.rope_base ** (-j / 32.0)
    pos = np.arange(c.T, dtype=np.float64)
    f = pos[:, None] * invf[None, :]
    cosT = np.cos(f).T.astype(np.float32)
    sinT = np.sin(f).T.astype(np.float32)
    cos64 = np.concatenate([cosT, cosT], 0)
    sin64 = np.concatenate([-sinT, sinT], 0)   # sign-folded rotate-half
    cos2 = np.concatenate([cos64, cos64], 0).astype(ml_dtypes.bfloat16)
    sin2 = np.concatenate([sin64, sin64], 0).astype(ml_dtypes.bfloat16)
    r = np.arange(128)[:, None]
    cc = np.arange(c.MASKW)[None, :]
    maskt = np.where(cc >= r + (c.NC - 128), 0.0, -1e30).astype(np.float32)
    return cos2, sin2, maskt


def make_core_inputs(c: Cfg, x, q_a_w, q_a_norm_w, q_b_w, kv_a_w, kv_norm_w,
                     kv_b_w, o_w, batch, heads, chunk):
    cos2, sin2, maskt = make_tables(c)
    scale = 1.0 / math.sqrt(c.QH)
    w1f = np.concatenate([q_a_w, kv_a_w], axis=1)          # [D, W1C]
    W1C = w1f.shape[1]
    NM1 = (W1C + 127) // 128
    KD = c.D // 128
    w1p = np.zeros((NM1 * 128, KD * 128), dtype=np.float32)
    for mi in range(NM1):
        m0 = mi * 128
        msz = min(128, W1C - m0)
        blk = w1f[:, m0:m0 + msz].reshape(KD, 128, msz).transpose(1, 0, 2)
        w1p[mi * 128:(mi + 1) * 128].reshape(128, KD, 128)[:, :, :msz] = blk
    w1 = np.ascontiguousarray(w1p)
    xT = np.ascontiguousarray(x[batch][chunk * c.TC:(chunk + 1) * c.TC, :].T)
    qb = q_b_w.reshape(c.QL, -1, c.QH)
    qbw = np.concatenate([qb[:, h, :c.NOPE] for h in heads] +
                         [qb[:, h, c.NOPE:] for h in heads], axis=1)
    qbw = np.ascontiguousarray(qbw * q_a_norm_w[:, None] * scale)
    kvb = kv_b_w.reshape(c.KVL, -1, c.NOPE + c.V)
    kbw = np.concatenate([kvb[:, h, :c.NOPE] for h in heads], axis=1)
    kbw = np.ascontiguousarray(kbw * kv_norm_w[:, None])
    vbw = np.concatenate([kvb[:, h, c.NOPE:] for h in heads], axis=1)
    vbw = np.ascontiguousarray(vbw * kv_norm_w[:, None])
    o3 = o_w.reshape(-1, c.V, c.D)
    ows = np.ascontiguousarray(np.concatenate([o3[h] for h in heads], axis=0))
    bf = ml_dtypes.bfloat16
    return {'xT': xT.astype(bf), 'w1': w1.astype(bf), 'qbw': qbw.astype(bf),
            'kbw': kbw.astype(bf), 'vbw': vbw.astype(bf), 'ow': ows.astype(bf),
            'cos2': cos2, 'sin2': sin2, 'maskt': maskt}


def prepare_in_maps(x, q_a_w, q_a_norm_w, q_b_w, kv_a_w, kv_norm_w, kv_b_w, o_w):
    args = [np.asarray(a, dtype=np.float32) for a in
            (x, q_a_w, q_a_norm_w, q_b_w, kv_a_w, kv_norm_w, kv_b_w, o_w)]
    in_maps = []
    for core in range(NCORES):
        b, g = core // GROUPS, core % GROUPS
        heads = list(range(g * NHC, (g + 1) * NHC))
        in_maps.append(make_core_inputs(FULL, *args, b, heads, g))
    return in_maps


def combine(results):
    out = np.zeros((B, T, D), dtype=np.float32)
    for core in range(NCORES):
        out[core // GROUPS] += results[core]['outT'].T.astype(np.float32)
    return out


_NC_CACHE = None


def kernel(x, q_a_w, q_a_norm_w, q_b_w, kv_a_w, kv_norm_w, kv_b_w, o_w):
    global _NC_CACHE
    in_maps = prepare_in_maps(x, q_a_w, q_a_norm_w, q_b_w, kv_a_w, kv_norm_w,
                              kv_b_w, o_w)
    if _NC_CACHE is None:
        _NC_CACHE = build_nc()
    res = run_bass_kernel_spmd(_NC_CACHE, in_maps, core_ids=list(range(NCORES)))
    return combine(res.results)



# revision 16
# speedup vs baseline: 1.0360x; 1.0360x over previous
"""MLA multi-head latent attention kernel for 8 TRN2 NeuronCores.

Sharding: 8 cores = 2 batches (DP) x 4 head-groups of 4 heads (TP).
The shared LoRA down-projection (x @ [q_a|kv_a]) is token-split across the
4 cores of each batch group (each computes its own 512-token chunk) and the
bf16 activations are AllGathered within the group — no replicated phase-1
work.  Three staggered gathers — [kv | q0], [q1..6], [q7..11] — overlap the
NRT collective latency (~35us trigger + serialization per op) with the
remaining phase-1 / kv_b / partial-q_b compute: q chunk 0 rides the kv
gather's mandatory slot, and each successive gather extends the q_b
accumulation chains (k<1, then k<7) so only ~5 contraction steps per chain
remain when the last gather lands.  A dummy warmup collective at kernel
start absorbs comm bring-up.

Key device facts this kernel is shaped around: the PE has DVFS p-states
(2.4 GHz only after ~3us of continuous busy, 1.2 GHz after any stall), so
every phase is software-pipelined to keep the PE gap-free; vector
reciprocal is ~16x slow, so 1/x and 1/sqrt run as scalar exp(-ln) /
exp(-0.5 ln); DMA-engine contention delays eviction DMAs (and therefore
collective triggers), so w1 is host-pre-arranged into contiguous per-chunk
tiles and later-phase weights prefetch only after the phase-1 stream.

On-device everything is feature-major ([feature, T]); no activation
transposes anywhere:
  - host supplies x pre-transposed per chunk (xT [D, 512]) and weights
    pre-sliced, with RMS-norm weights and softmax scale folded into the
    up-projections
  - kv_c is RMS-normalized locally in phase 1 (exact f32 sum-of-squares off
    PSUM via ones-matmul) and gathered pre-scaled, so kv_b needs no norm
    handling downstream; q is gathered raw and its norm recomputed per
    chunk (ones-matmul trick), applied at PSUM-eviction time
  - q up-projection outputs stay SBUF-resident ([768, T] bf16) through
    attention (no DRAM staging)
  - attention computes scores transposed ([k, q]) with a lookahead-2
    software pipeline (scores of tile i+2 issue before attn@v of tile i, so
    the PE never waits on the scalar exp); the softmax-sum accumulates on
    the vector engine (frees a PSUM bank and one PE pass per tile) with a
    single ones-matmul partition-reduce per head; exp runs without
    max-subtraction (scores are small by construction)
  - o_proj for chunk qn-1 is emitted during attention of chunk qn (absorbs
    eviction latency at chunk boundaries); partial outputs are written f16
    ([D, T]) and summed on host
Measured: 619 us on hardware under a heavily power-throttled device state
(util limit 0.68; the prior 3-gather structure measured 627-694 us under
comparable throttle; baseline replicated-phase-1 version was 702 us),
rel err 5.9e-3 vs the fp32 reference.
"""
import math
import sys
from collections import deque
from contextlib import ExitStack
from dataclasses import dataclass

sys.path.insert(0, '/opt/trn_rl_repo')
import numpy as np
import ml_dtypes
import concourse.bass as bass
import concourse.bacc as bacc
import concourse.mybir as mybir
from concourse import tile
from concourse.bass_utils import run_bass_kernel_spmd

F32 = mybir.dt.float32
F16 = mybir.dt.float16
BF16 = mybir.dt.bfloat16
AF = mybir.ActivationFunctionType


@dataclass
class Cfg:
    T: int = 2048
    D: int = 2048
    QL: int = 1536
    KVL: int = 512
    NHC: int = 4          # heads per core
    NOPE: int = 128
    ROPE: int = 64
    V: int = 128
    G: int = 4            # cores per TP group = token chunks
    eps: float = 1e-6
    rope_base: float = 10000.0

    @property
    def TC(self):         # tokens computed locally in phase 1
        return self.T // self.G

    @property
    def NC(self):
        return min(512, self.T)

    @property
    def KD(self):
        return self.D // 128

    @property
    def MQ(self):
        return self.QL // 128

    @property
    def MKV(self):
        return self.KVL // 128

    @property
    def NT(self):
        return self.T // self.NC

    @property
    def TK(self):
        return self.T // 128

    @property
    def MASKW(self):
        return 2 * self.NC - 128

    @property
    def QH(self):
        return self.NOPE + self.ROPE

    @property
    def NQM(self):        # q_b output 128-chunks
        return (self.NHC * self.QH) // 128

    @property
    def KVC(self):        # kv_a output rows (latent + rope)
        return self.KVL + self.ROPE


# full-scale problem constants (per harness contract)
B, T, D = 2, 2048, 2048
QL, KVL = 1536, 512
NHEADS, NOPE, ROPE, V = 16, 128, 64, 128
QH = NOPE + ROPE
NCORES = 8
GROUPS = 4
NHC = NHEADS // GROUPS
FULL = Cfg()
RG = [[0, 1, 2, 3], [4, 5, 6, 7]]
MQ1 = 0               # q chunks riding in the first (kv) gather: none — the
                      # kv gather must trigger as soon as kv+rope evict (~35us)
                      # since the first collective pays ~45us NRT bring-up
MQ2 = 7               # first q chunk of the third gather


def _r(ap):
    return ap


def build_nc(c: Cfg = FULL, num_devices: int = NCORES):
    nc = bacc.Bacc("TRN2", target_bir_lowering=False, debug=False,
                   num_devices=num_devices)
    W1C = c.QL + c.KVL + c.ROPE
    TC = c.TC

    NM1 = (W1C + 127) // 128
    xT = nc.dram_tensor("xT", [c.D, TC], BF16, kind="ExternalInput").ap()
    # w1 pre-arranged on host: chunk mi's [128, KD*128] tile is contiguous
    w1 = nc.dram_tensor("w1", [NM1 * 128, c.KD * 128], BF16,
                        kind="ExternalInput").ap()
    qbw = nc.dram_tensor("qbw", [c.QL, c.NHC * c.QH], BF16, kind="ExternalInput").ap()
    kbw = nc.dram_tensor("kbw", [c.KVL, c.NHC * 128], BF16, kind="ExternalInput").ap()
    vbw = nc.dram_tensor("vbw", [c.KVL, c.NHC * c.V], BF16, kind="ExternalInput").ap()
    ow = nc.dram_tensor("ow", [c.NHC * c.V, c.D], BF16, kind="ExternalInput").ap()
    cos2 = nc.dram_tensor("cos2", [128, c.T], BF16, kind="ExternalInput").ap()
    sin2 = nc.dram_tensor("sin2", [128, c.T], BF16, kind="ExternalInput").ap()
    maskt = nc.dram_tensor("maskt", [128, c.MASKW], F32, kind="ExternalInput").ap()
    outT = nc.dram_tensor("outT", [c.D, c.T], F16, kind="ExternalOutput").ap()

    m1 = []
    off = 0
    while off < W1C:
        sz = min(128, W1C - off)
        m1.append((off, sz))
        off += sz
    m_order = list(range(c.MQ, len(m1))) + list(range(c.MQ))  # kv chunks first

    with tile.TileContext(nc) as tc, ExitStack() as top:
        # gather 1 carries kv + the first MQ1 q chunks (asymmetric split:
        # the chain [warmup -> kv+q0..2 -> q3..11] finishes ~30us earlier
        # than [kv -> q0..5 -> q6..11] and the k<MQ1 partial q_b chains fill
        # the second transfer window)
        KQ1 = c.KVC + MQ1 * 128
        dram = top.enter_context(tc.tile_pool(name="dram", bufs=1, space="DRAM"))
        # each DRAM tile gets its own tag: untagged tiles share one tag-meta
        # rotation in the pool, which serializes later tiles (and the gathers
        # writing them) behind every reader of the earlier ones
        kq_loc = dram.tile([KQ1, TC], BF16, tag="kq_loc")
        q2_loc = dram.tile([(MQ2 - MQ1) * 128, TC], BF16, tag="q2_loc")
        q3_loc = dram.tile([(c.MQ - MQ2) * 128, TC], BF16, tag="q3_loc")
        kqg = dram.tile([c.NT * KQ1, TC], BF16, tag="kqg")
        qg2 = dram.tile([c.NT * (MQ2 - MQ1) * 128, TC], BF16, tag="qg2")
        qg3 = dram.tile([c.NT * (c.MQ - MQ2) * 128, TC], BF16, tag="qg3")
        # (no warmup collective: the NRT start barrier covers comm bring-up,
        # and a warmup op only serializes ahead of the first real gather)

        const = top.enter_context(tc.tile_pool(name="const", bufs=1))
        ones_f = const.tile([128, 128], F32)
        nc.vector.memset(ones_f[:], 1.0)
        ones = const.tile([128, 128], BF16)
        nc.vector.tensor_copy(ones[:], ones_f[:])
        eps_sb = const.tile([128, 1], F32)
        nc.vector.memset(eps_sb[:], float(c.eps))
        cos_sb = const.tile([128, c.T], BF16, tag="cos")
        sin_sb = const.tile([128, c.T], BF16, tag="sin")
        # later-phase weight pools opened early: their loads are issued
        # mid-phase-1 (after the kv gather fires) so the gather windows have
        # the DMA engines to themselves
        kw_pool = top.enter_context(tc.tile_pool(name="kw", bufs=1))
        kbw_sb = [kw_pool.tile([128, c.NHC * 128], BF16, tag=f"kbw{k}",
                               name=f"kbw{k}") for k in range(c.MKV)]
        vbw_sb = [kw_pool.tile([128, c.NHC * c.V], BF16, tag=f"vbw{k}",
                               name=f"vbw{k}") for k in range(c.MKV)]
        qw_pool = top.enter_context(tc.tile_pool(name="qw", bufs=1))
        qbw_sb = [qw_pool.tile([128, c.NHC * c.QH], BF16, tag=f"qbw{k}",
                               name=f"qbw{k}") for k in range(c.MQ)]
        mk_pool = top.enter_context(tc.tile_pool(name="mask", bufs=1))
        mask_sb = mk_pool.tile([128, c.MASKW], F32)

        kvc = top.enter_context(tc.tile_pool(name="kvc", bufs=1))  # "KV cache"
        knope = [kvc.tile([128, c.T], BF16, tag=f"kn{i}", name=f"kn{i}")
                 for i in range(c.NHC)]
        krope = kvc.tile([128, c.T], BF16, tag="krope")  # duplicated halves
        vsb = [kvc.tile([128, c.NHC * c.V], BF16, tag=f"v{i}", name=f"v{i}")
               for i in range(c.TK)]
        qsb = [kvc.tile([128, c.T], BF16, tag=f"q{m}", name=f"qs{m}")
               for m in range(c.NQM)]   # q up-proj output, SBUF-resident

        # ---------------- phase 1: local-chunk x @ [q_a | kv_a | k_rope] ----
        with ExitStack() as p1:
            xt_pool = p1.enter_context(tc.tile_pool(name="xt", bufs=1))
            w1_pool = p1.enter_context(tc.tile_pool(name="w1", bufs=2))
            ev_pool = p1.enter_context(tc.tile_pool(name="p1ev", bufs=18))
            ps_pool = p1.enter_context(tc.tile_pool(name="p1ps", bufs=2, space="PSUM"))
            kssq_ps = p1.enter_context(tc.tile_pool(name="kssq", bufs=1, space="PSUM"))
            krs_pool = p1.enter_context(tc.tile_pool(name="krs", bufs=1))

            qssq_ps = p1.enter_context(tc.tile_pool(name="qssq", bufs=1,
                                                    space="PSUM"))
            xt_sb = [xt_pool.tile([128, TC], BF16, tag=f"xt{k}", name=f"xt{k}")
                     for k in range(c.KD)]

            def load_wt(mi):
                wt = w1_pool.tile([128, c.KD, 128], BF16, tag="w1t")
                src = w1[mi * 128:(mi + 1) * 128, :].rearrange(
                    "p (k c) -> p k c", k=c.KD)
                nc.sync.dma_start(wt[:], src)
                return wt

            # first weight chunk before the xt stream so chain 0 starts early
            wts = {m_order[0]: load_wt(m_order[0])}
            for k in range(c.KD):
                nc.sync.dma_start(xt_sb[k][:], xT[k * 128:(k + 1) * 128, :])
            # kv_c AND q are RMS-normalized locally (exact f32 ssq from PSUM)
            # and gathered pre-scaled, so neither kv_b nor q_b needs any norm
            # handling downstream (the q ssq-recompute used to cost 48 PE MMs
            # + 48 vector muls + 8 Ln/Exp table switches post-gather).
            kv_evs, q_evs = [], []
            kssq = kssq_ps.tile([128, TC], F32, tag="kssq", name="kssq")
            qssq = qssq_ps.tile([128, TC], F32, tag="qssq", name="qssq")
            for pos, mi in enumerate(m_order):
                m0, msz = m1[mi]
                wt = wts.pop(mi) if mi in wts else load_wt(mi)
                ps = ps_pool.tile([128, TC], F32, tag="ps")
                for k in range(c.KD):
                    nc.tensor.matmul(ps[:msz, :], _r(wt[:, k, :msz]),
                                     _r(xt_sb[k][:]),
                                     start=(k == 0), stop=(k == c.KD - 1))
                ev = ev_pool.tile([128, TC], BF16, tag="ev")
                nc.scalar.copy(ev[:msz, :], ps[:msz, :])
                is_kvc = c.MQ <= mi < c.MQ + c.MKV
                if is_kvc:
                    sq = ev_pool.tile([128, TC], BF16, tag="sq1", name="sq1")
                    nc.scalar.square(sq[:], ps[:])
                    nc.tensor.matmul(kssq[:], _r(ones[:]), _r(sq[:]),
                                     start=(mi == c.MQ),
                                     stop=(mi == c.MQ + c.MKV - 1))
                    kv_evs.append((m0 - c.QL, msz, ev))
                    if mi == c.MQ + c.MKV - 1:
                        lt = krs_pool.tile([128, TC], F32, tag="klt", name="klt")
                        krsq = krs_pool.tile([128, TC], F32, tag="krsq",
                                             name="krsq")
                        nc.scalar.activation(lt[:], kssq[:], AF.Ln,
                                             bias=eps_sb[:], scale=1.0 / c.KVL)
                        nc.scalar.activation(krsq[:], lt[:], AF.Exp, scale=-0.5)
                        for (roff, msze, eve) in kv_evs:
                            nc.vector.tensor_mul(eve[:msze, :], eve[:msze, :],
                                                 krsq[:msze, :])
                            nc.sync.dma_start(kq_loc[roff:roff + msze, :],
                                              eve[:msze, :])
                    continue
                if mi >= c.MQ:          # rope chunk: evict raw (no norm)
                    nc.sync.dma_start(kq_loc[m0 - c.QL:m0 - c.QL + msz, :],
                                      ev[:msz, :])
                    # kv + rope fully evicted: fire the kv gather now
                    nc.gpsimd.collective_compute(
                        "AllGather", mybir.AluOpType.bypass,
                        replica_groups=RG,
                        ins=[kq_loc.opt()], outs=[kqg.opt()])
                    # weights needed right after this gather load now,
                    # before the transfer windows occupy the DMA engines
                    nc.sync.dma_start(cos_sb[:], cos2[:])
                    nc.sync.dma_start(sin_sb[:], sin2[:])
                    nc.sync.dma_start(mask_sb[:], maskt[:])
                    for k in range(c.MKV):
                        nc.sync.dma_start(kbw_sb[k][:],
                                          kbw[k * 128:(k + 1) * 128, :])
                        nc.sync.dma_start(vbw_sb[k][:],
                                          vbw[k * 128:(k + 1) * 128, :])
                    continue
                # q chunk: accumulate ssq, hold the eviction until qrsq ready
                sq = ev_pool.tile([128, TC], BF16, tag="sq1", name="sq1")
                nc.scalar.square(sq[:], ps[:])
                nc.tensor.matmul(qssq[:], _r(ones[:]), _r(sq[:]),
                                 start=(mi == 0), stop=(mi == c.MQ - 1))
                q_evs.append((mi, msz, ev))
                if mi != c.MQ - 1:
                    continue
                qlt = krs_pool.tile([128, TC], F32, tag="qlt", name="qlt")
                qrsq = krs_pool.tile([128, TC], F32, tag="qrsq", name="qrsq")
                nc.scalar.activation(qlt[:], qssq[:], AF.Ln,
                                     bias=eps_sb[:], scale=1.0 / c.QL)
                nc.scalar.activation(qrsq[:], qlt[:], AF.Exp, scale=-0.5)
                for (qmi, msze, eve) in q_evs:
                    nc.vector.tensor_mul(eve[:msze, :], eve[:msze, :],
                                         qrsq[:msze, :])
                    if qmi < MQ1:
                        dst, roff = kq_loc, c.KVC + qmi * 128
                    elif qmi < MQ2:
                        dst, roff = q2_loc, (qmi - MQ1) * 128
                    else:
                        dst, roff = q3_loc, (qmi - MQ2) * 128
                    nc.sync.dma_start(dst[roff:roff + msze, :], eve[:msze, :])
                    if qmi == MQ2 - 1:
                        nc.gpsimd.collective_compute(
                            "AllGather", mybir.AluOpType.bypass,
                            replica_groups=RG,
                            ins=[q2_loc.opt()], outs=[qg2.opt()])
                    elif qmi == c.MQ - 1:
                        nc.gpsimd.collective_compute(
                            "AllGather", mybir.AluOpType.bypass,
                            replica_groups=RG,
                            ins=[q3_loc.opt()], outs=[qg3.opt()])
                for k in range(c.MQ):
                    nc.sync.dma_start(qbw_sb[k][:],
                                      qbw[k * 128:(k + 1) * 128, :])

        # ---------------- phases 2a (kv_b) + 2b (q_b) ----------------
        with ExitStack() as mid:
            kvr_pool = mid.enter_context(tc.tile_pool(name="kvr", bufs=1))
            kv_raw = [kvr_pool.tile([128, c.T], BF16, tag=f"kvr{k}", name=f"kvr{k}")
                      for k in range(c.MKV)]
            for n in range(c.NT):
                ns = slice(n * c.NC, (n + 1) * c.NC)
                base = n * KQ1
                for k in range(c.MKV):
                    nc.sync.dma_start(kv_raw[k][:, ns],
                                      kqg[base + k * 128:base + (k + 1) * 128, :])
                nc.sync.dma_start(krope[0:64, ns],
                                  kqg[base + c.KVL:base + c.KVC, :])
                nc.sync.dma_start(krope[64:128, ns],
                                  kqg[base + c.KVL:base + c.KVC, :])
            rt_pool = mid.enter_context(tc.tile_pool(name="rt", bufs=2))
            ps2 = mid.enter_context(tc.tile_pool(name="ps2", bufs=2, space="PSUM"))

            def rope_cols(x_ap, rows, ns):
                # in-place rotate-half on [rows, NC] slice; tables sliced to ns
                tmp = rt_pool.tile([128, c.NC], BF16, tag="rtmp", name="rtmp")
                t1 = rt_pool.tile([128, c.NC], BF16, tag="rt1", name="rt1")
                for b0 in range(0, rows, 64):
                    nc.sync.dma_start(tmp[b0:b0 + 32, :], x_ap[b0 + 32:b0 + 64, :])
                    nc.sync.dma_start(tmp[b0 + 32:b0 + 64, :], x_ap[b0:b0 + 32, :])
                nc.vector.tensor_mul(tmp[:rows, :], tmp[:rows, :],
                                     sin_sb[:rows, ns])
                nc.vector.tensor_mul(t1[:rows, :], x_ap[:rows, :],
                                     cos_sb[:rows, ns])
                nc.vector.tensor_add(x_ap[:rows, :], t1[:rows, :], tmp[:rows, :])

            for n in range(c.NT):
                ns = slice(n * c.NC, (n + 1) * c.NC)
                rope_cols(krope[:, ns], 128, ns)

            # gathered kv_c is already RMS-normalized: plain evictions
            for h in range(c.NHC):
                for n in range(c.NT):
                    ns = slice(n * c.NC, (n + 1) * c.NC)
                    ps = ps2.tile([128, c.NC], F32, tag="ps")
                    for k in range(c.MKV):
                        nc.tensor.matmul(ps[:], _r(kbw_sb[k][:, h * 128:(h + 1) * 128]),
                                         _r(kv_raw[k][:, ns]),
                                         start=(k == 0), stop=(k == c.MKV - 1))
                    nc.vector.tensor_copy(knope[h][:, ns], ps[:])

            for m in range(c.TK):
                ms = slice(m * 128, (m + 1) * 128)
                ps = ps2.tile([128, c.NC], F32, tag="ps")
                for k in range(c.MKV):
                    nc.tensor.matmul(ps[:], _r(kv_raw[k][:, ms]),
                                     _r(vbw_sb[k][:]),
                                     start=(k == 0), stop=(k == c.MKV - 1))
                nc.vector.tensor_copy(vsb[m][:], ps[:])

            # ------------ phase 2b: q_b (+ RoPE on q), SBUF-resident --------
            xq_pool = mid.enter_context(tc.tile_pool(name="xq", bufs=2 * c.MQ + 2))
            # 6 banks: while AG3 is in flight, all six of a chunk's q_b chains
            # can advance through the k<MQ2 prefix and park at k=MQ2 holding
            # their bank (with 2 banks only 2 chains park -> 22us PE idle)
            ps2b = mid.enter_context(tc.tile_pool(name="ps2b", bufs=6, space="PSUM"))

            for n in range(c.NT):
                ns = slice(n * c.NC, (n + 1) * c.NC)
                xq = []
                for k in range(c.MQ):
                    t = xq_pool.tile([128, c.NC], BF16, tag="xq", name=f"xq{k}")
                    if k < MQ1:
                        base = n * KQ1 + c.KVC + k * 128
                        src = kqg[base:base + 128, :]
                    elif k < MQ2:
                        base = n * (MQ2 - MQ1) * 128 + (k - MQ1) * 128
                        src = qg2[base:base + 128, :]
                    else:
                        base = n * (c.MQ - MQ2) * 128 + (k - MQ2) * 128
                        src = qg3[base:base + 128, :]
                    nc.sync.dma_start(t[:], src)
                    xq.append(t)
                # gathered q is already RMS-normalized: plain evictions
                for m in range(c.NQM):
                    ps = ps2b.tile([128, c.NC], F32, tag="ps")
                    for k in range(c.MQ):
                        nc.tensor.matmul(ps[:], _r(qbw_sb[k][:, m * 128:(m + 1) * 128]),
                                         _r(xq[k][:]),
                                         start=(k == 0), stop=(k == c.MQ - 1))
                    nc.vector.tensor_copy(qsb[m][:, ns], ps[:])
                    if m >= c.NHC:
                        rope_cols(qsb[m][:, ns], 128, ns)

        # ---------------- phase 3+4: attention + interleaved o_proj --------
        with ExitStack() as late:
            av_pool = late.enter_context(tc.tile_pool(name="avt", bufs=1))
            avt = [av_pool.tile([128, c.T], BF16, tag=f"av{i}", name=f"av{i}")
                   for i in range(c.NHC)]
            ow_pool = late.enter_context(tc.tile_pool(name="ow", bufs=1))
            ow_sb = [ow_pool.tile([128, c.D], BF16, tag=f"ow{h}", name=f"ow{h}")
                     for h in range(c.NHC)]
            for h in range(c.NHC):
                nc.sync.dma_start(ow_sb[h][:], ow[h * c.V:(h + 1) * c.V, :])

            s_ps = late.enter_context(tc.tile_pool(name="sps", bufs=3, space="PSUM"))
            av_ps = late.enter_context(tc.tile_pool(name="avps", bufs=2, space="PSUM"))
            sm_ps = late.enter_context(tc.tile_pool(name="smps", bufs=1, space="PSUM"))
            ps4 = late.enter_context(tc.tile_pool(name="ps4", bufs=2, space="PSUM"))
            e_pool = late.enter_context(tc.tile_pool(name="e", bufs=6))
            acc_pool = late.enter_context(tc.tile_pool(name="acc", bufs=2))
            rs_pool = late.enter_context(tc.tile_pool(name="rs", bufs=2))
            ev4 = late.enter_context(tc.tile_pool(name="ev4", bufs=3))

            def nkj_of(qn):
                return ((qn + 1) * c.NC) // 128

            def score(qn, h, kj):
                # scores^T for one 128-key tile -> exp'd bf16 e tile
                qsl = slice(qn * c.NC, (qn + 1) * c.NC)
                qr_t = qsb[c.NHC + (h * c.ROPE) // 128]
                qr_r0 = (h * c.ROPE) % 128
                ks = slice(kj * 128, (kj + 1) * 128)
                pss = s_ps.tile([128, c.NC], F32, tag="pss")
                nc.tensor.matmul(pss[:], _r(knope[h][:, ks]),
                                 _r(qsb[h][:, qsl]), start=True, stop=False)
                nc.tensor.matmul(pss[:], _r(krope[qr_r0:qr_r0 + 64, ks]),
                                 _r(qr_t[qr_r0:qr_r0 + 64, qsl]),
                                 start=False, stop=True)
                e = e_pool.tile([128, c.NC], BF16, tag="e")
                off = kj * 128 - qn * c.NC
                if off >= 0:  # diagonal tile: causal mask
                    msl = mask_sb[:, c.NC - 128 - off:2 * c.NC - 128 - off]
                    nc.vector.tensor_add(e[:], pss[:], msl)
                    nc.scalar.activation(e[:], e[:], AF.Exp)
                else:
                    nc.scalar.activation(e[:], pss[:], AF.Exp)
                return e

            cur = {}           # (qn, h) -> (pav psum, acc sbuf)
            key_order = []

            def avsum(qn, h, kj, e):
                # attn@v on PE; softmax-sum accumulated on vector (frees a
                # PSUM bank and 1 PE pass per tile)
                if kj == 0:
                    cur[(qn, h)] = (av_ps.tile([128, c.NC], F32, tag="pav",
                                               name="pav"),
                                    acc_pool.tile([128, c.NC], F32, tag="acc",
                                                  name="acc"))
                    key_order.append((qn, h))
                pav, acc = cur[(qn, h)]
                first, last = (kj == 0), (kj == nkj_of(qn) - 1)
                nc.tensor.matmul(pav[:], _r(vsb[kj][:, h * c.V:(h + 1) * c.V]),
                                 _r(e[:]), start=first, stop=last)
                if first:
                    nc.vector.tensor_copy(acc[:], e[:])
                else:
                    nc.vector.tensor_add(acc[:], acc[:], e[:])

            def finalize(qn, h):
                # partition-reduce the vector-accumulated sum with one
                # ones-matmul, then 1/sum on the vector engine (keeps Ln off
                # the scalar queue: any Ln between attention Exps forces a
                # ~2.7us ACT_TABLE_LOAD pair that stalls the exp->attn@v path)
                qsl = slice(qn * c.NC, (qn + 1) * c.NC)
                pav, acc = cur.pop((qn, h))
                ab = e_pool.tile([128, c.NC], BF16, tag="accb", name="accb")
                nc.vector.tensor_copy(ab[:], acc[:])
                psm = sm_ps.tile([128, c.NC], F32, tag="psm", name="psm")
                nc.tensor.matmul(psm[:], _r(ones[:]), _r(ab[:]),
                                 start=True, stop=True)
                rs = rs_pool.tile([128, c.NC], F32, tag="rs")
                nc.vector.reciprocal_approx_fast(rs[:], psm[:])
                nc.vector.tensor_mul(avt[h][:, qsl], pav[:], rs[:])

            def oproj(qn):
                qsl = slice(qn * c.NC, (qn + 1) * c.NC)
                for m in range(c.KD):
                    ms = slice(m * 128, (m + 1) * 128)
                    ps = ps4.tile([128, c.NC], F32, tag="ps")
                    for h in range(c.NHC):
                        nc.tensor.matmul(ps[:], _r(ow_sb[h][:, ms]),
                                         _r(avt[h][:, qsl]),
                                         start=(h == 0), stop=(h == c.NHC - 1))
                    ev = ev4.tile([128, c.NC], F16, tag="ev")
                    # scalar, not vector: the vector queue is the co-critical
                    # engine during attention (acc adds); ScE reads PSUM fast
                    nc.scalar.copy(ev[:], ps[:])
                    nc.sync.dma_start(outT[ms, qsl], ev[:])

            # software pipeline with lookahead 2: PE issues scores two tiles
            # ahead of attn@v, so the scalar exp latency is fully hidden and
            # the PE stream never bubbles (keeps the DVFS p-state at max).
            # o_proj for chunk qn-1 is emitted during chunk qn to absorb the
            # eviction latency at chunk boundaries.
            pend = deque()
            done_keys = 0

            def pop_one():
                nonlocal done_keys
                (pqn, ph, pkj, pe) = pend.popleft()
                avsum(pqn, ph, pkj, pe)
                # finalize the previous head once the next head is 4 tiles in
                # (gives the vector queue time to drain the sum accumulation)
                if pkj == 3 and done_keys < len(key_order) - 1:
                    finalize(*key_order[done_keys])
                    done_keys += 1

            for qn in range(c.NT):
                for h in range(c.NHC):
                    for kj in range(nkj_of(qn)):
                        e = score(qn, h, kj)
                        pend.append((qn, h, kj, e))
                        if len(pend) > 3:
                            pop_one()
                if qn > 0:
                    oproj(qn - 1)
            while pend:
                pop_one()
            while done_keys < len(key_order):
                finalize(*key_order[done_keys])
                done_keys += 1
            oproj(c.NT - 1)

    nc.compile()
    return nc


# ---------------- host-side prep ----------------
def make_tables(c: Cfg):
    j = np.arange(32, dtype=np.float64)
    invf = c.rope_base ** (-j / 32.0)
    pos = np.arange(c.T, dtype=np.float64)
    f = pos[:, None] * invf[None, :]
    cosT = np.cos(f).T.astype(np.float32)
    sinT = np.sin(f).T.astype(np.float32)
    cos64 = np.concatenate([cosT, cosT], 0)
    sin64 = np.concatenate([-sinT, sinT], 0)   # sign-folded rotate-half
    cos2 = np.concatenate([cos64, cos64], 0).astype(ml_dtypes.bfloat16)
    sin2 = np.concatenate([sin64, sin64], 0).astype(ml_dtypes.bfloat16)
    r = np.arange(128)[:, None]
    cc = np.arange(c.MASKW)[None, :]
    maskt = np.where(cc >= r + (c.NC - 128), 0.0, -1e30).astype(np.float32)
    return cos2, sin2, maskt


def make_core_inputs(c: Cfg, x, q_a_w, q_a_norm_w, q_b_w, kv_a_w, kv_norm_w,
                     kv_b_w, o_w, batch, heads, chunk):
    cos2, sin2, maskt = make_tables(c)
    scale = 1.0 / math.sqrt(c.QH)
    w1f = np.concatenate([q_a_w, kv_a_w], axis=1)          # [D, W1C]
    W1C = w1f.shape[1]
    NM1 = (W1C + 127) // 128
    KD = c.D // 128
    w1p = np.zeros((NM1 * 128, KD * 128), dtype=np.float32)
    for mi in range(NM1):
        m0 = mi * 128
        msz = min(128, W1C - m0)
        blk = w1f[:, m0:m0 + msz].reshape(KD, 128, msz).transpose(1, 0, 2)
        w1p[mi * 128:(mi + 1) * 128].reshape(128, KD, 128)[:, :, :msz] = blk
    w1 = np.ascontiguousarray(w1p)
    xT = np.ascontiguousarray(x[batch][chunk * c.TC:(chunk + 1) * c.TC, :].T)
    qb = q_b_w.reshape(c.QL, -1, c.QH)
    qbw = np.concatenate([qb[:, h, :c.NOPE] for h in heads] +
                         [qb[:, h, c.NOPE:] for h in heads], axis=1)
    qbw = np.ascontiguousarray(qbw * q_a_norm_w[:, None] * scale)
    kvb = kv_b_w.reshape(c.KVL, -1, c.NOPE + c.V)
    kbw = np.concatenate([kvb[:, h, :c.NOPE] for h in heads], axis=1)
    kbw = np.ascontiguousarray(kbw * kv_norm_w[:, None])
    vbw = np.concatenate([kvb[:, h, c.NOPE:] for h in heads], axis=1)
    vbw = np.ascontiguousarray(vbw * kv_norm_w[:, None])
    o3 = o_w.reshape(-1, c.V, c.D)
    ows = np.ascontiguousarray(np.concatenate([o3[h] for h in heads], axis=0))
    bf = ml_dtypes.bfloat16
    return {'xT': xT.astype(bf), 'w1': w1.astype(bf), 'qbw': qbw.astype(bf),
            'kbw': kbw.astype(bf), 'vbw': vbw.astype(bf), 'ow': ows.astype(bf),
            'cos2': cos2, 'sin2': sin2, 'maskt': maskt}


def prepare_in_maps(x, q_a_w, q_a_norm_w, q_b_w, kv_a_w, kv_norm_w, kv_b_w, o_w):
    args = [np.asarray(a, dtype=np.float32) for a in
            (x, q_a_w, q_a_norm_w, q_b_w, kv_a_w, kv_norm_w, kv_b_w, o_w)]
    in_maps = []
    for core in range(NCORES):
        b, g = core // GROUPS, core % GROUPS
        heads = list(range(g * NHC, (g + 1) * NHC))
        in_maps.append(make_core_inputs(FULL, *args, b, heads, g))
    return in_maps


def combine(results):
    out = np.zeros((B, T, D), dtype=np.float32)
    for core in range(NCORES):
        out[core // GROUPS] += results[core]['outT'].T.astype(np.float32)
    return out


_NC_CACHE = None


def kernel(x, q_a_w, q_a_norm_w, q_b_w, kv_a_w, kv_norm_w, kv_b_w, o_w):
    global _NC_CACHE
    in_maps = prepare_in_maps(x, q_a_w, q_a_norm_w, q_b_w, kv_a_w, kv_norm_w,
                              kv_b_w, o_w)
    if _NC_CACHE is None:
        _NC_CACHE = build_nc()
    res = run_bass_kernel_spmd(_NC_CACHE, in_maps, core_ids=list(range(NCORES)))
    return combine(res.results)

